# revision 36
# baseline (speedup 1.0000x reference)
"""Dense-CRF relaxed Potts loss on 8 TRN2 NeuronCores — triangle version.

v2: exploits W's symmetry to nearly halve the exp work (the v1 bottleneck).

Math: loss*N = sum_ij s_i W_ij (1-s_j).  Split the 72x72 grid of 128-row
slab pairs by cyclic offset d = (b-a) mod 72:
  d=0   : self block, direct formula only
  1..35 : process pair once; direct + mirrored contribution:
            direct_ij = s_i W_ij (1-s_j)     (ACT bias log s_i + accum_out)
            mirror_ij = (1-s_i) W_ij s_j = h_i * direct_ij * r_j
          with r_j = s_j/(1-s_j): DVE tensor_tensor T*R (bf16 2x mode), then
          PE contracts h^T (T*R) with a single PSUM accumulator [2,512]
          (h in 2 bf16 limbs; column-folded sums; final reduce on host)
  d=36  : antipodal pairs appear once for each of the two owning slabs ->
          direct formula only (both directions covered)
SPMD uniformity: core k owns slabs {k+8t}; its copy of the B/R data is
ROTATED by k slabs so the program's column offsets (8t+d) mod 72 are
core-independent.

The z matmul uses a K=36 bf16 3-limb decomposition (features, sq, and the
log column terms each split into bf16 limbs, cross products paired so that
sum_k a_k[i] b_k[j] = -0.5*d2_ij + log1p(-s_j) to ~2^-24) giving fp32-grade
d2 at the bf16 rate of 1 cycle/row — 4x faster than a native fp32 matmul.
"""

import numpy as np
import ml_dtypes

import concourse.bacc as bacc
import concourse.tile as tile
from concourse import mybir
import concourse.bass_utils as bass_utils

BF16 = ml_dtypes.bfloat16

SIGMA_XY = 15.0
SIGMA_RGB = 0.125
H = W = 96
N = H * W                   # 9216
N_CORES = 8
NSLAB = N // 128            # 72 slabs of 128 rows
T_SLABS = NSLAB // N_CORES  # 9 per core
D_MAX = 36                  # offsets 0..36
GROUP_CAPS = (16, 12)       # alternating PSUM group sizes (4 banks / 3 banks)

_cached = {}


def _slab_runs(t):
    """Column-slab runs (m0, length, d0) for local slab t (start m = 8t)."""
    m0 = 8 * t
    if m0 + D_MAX <= NSLAB - 1:
        return [(m0, D_MAX + 1, 0)]
    l1 = NSLAB - m0
    return [(m0, l1, 0), (0, D_MAX + 1 - l1, l1)]


def _groups():
    """Compile-time schedule: list of (t, mb0, nb, mir_lo, mir_hi, parity)
    where mir_lo/mir_hi are group-local block bounds of the mirror range and
    parity selects which of the two alternating PSUM slots the group uses."""
    out = []
    parity = 0
    for t in range(T_SLABS):
        for (m0, L, d0) in _slab_runs(t):
            blo = max(0, 1 - d0)           # run-local mirror block range
            bhi = min(L, D_MAX - d0)
            b0 = 0
            while b0 < L:
                nb = min(GROUP_CAPS[parity], L - b0)
                # keep the kernel's very last group mirror-free (d=36 block
                # alone) so the tail is ACT-only, not an ACT->DVE->PE chain
                if t == T_SLABS - 1 and b0 < bhi < b0 + nb:
                    nb = bhi - b0
                mlo = max(b0, blo) - b0
                mhi = min(b0 + nb, bhi) - b0
                out.append((t, m0 + b0, nb, max(mlo, 0), max(mhi, 0), parity))
                b0 += nb
                parity ^= 1
    return out


def _build_module():
    groups = _groups()
    n_accd = len(groups)

    nc = bacc.Bacc(
        "TRN2",
        target_bir_lowering=False,
        debug=False,
        enable_asserts=False,
        num_devices=N_CORES,
    )
    f32 = mybir.dt.float32
    bf = mybir.dt.bfloat16
    a_src = nc.dram_tensor("a_src", [36, T_SLABS * 128], bf, kind="ExternalInput").ap()
    b_src = nc.dram_tensor("b_src", [36, N], bf, kind="ExternalInput").ap()
    logs_src = nc.dram_tensor("logs_src", [128, T_SLABS], f32, kind="ExternalInput").ap()
    r_src = nc.dram_tensor("r_src", [1, N], bf, kind="ExternalInput").ap()
    h_src = nc.dram_tensor("h_src", [128, 2 * T_SLABS], bf, kind="ExternalInput").ap()
    accd_out = nc.dram_tensor("accd_out", [128, n_accd], f32, kind="ExternalOutput").ap()
    m2_out = nc.dram_tensor("m2_out", [2, 512], f32, kind="ExternalOutput").ap()

    # count mirror matmul chunks to set start/stop flags
    n_mir = 0
    for (t, mb0, nb, mlo, mhi, parity) in groups:
        if mhi > mlo:
            w = (mhi - mlo) * 128
            n_mir += (w + 511) // 512

    with tile.TileContext(nc) as tc:
        with (
            tc.tile_pool(name="singles", bufs=1) as singles,
            tc.tile_pool(name="psA", bufs=1, space="PSUM") as psA_pool,
            tc.tile_pool(name="psB", bufs=1, space="PSUM") as psB_pool,
            tc.tile_pool(name="m2ps", bufs=1, space="PSUM") as m2_pool,
            tc.tile_pool(name="tpool", bufs=3) as t_pool,
        ):
            A = singles.tile([36, T_SLABS * 128], bf)
            B = singles.tile([36, N], bf)
            R = singles.tile([128, N], bf)
            LOGS = singles.tile([128, T_SLABS], f32)
            Hh = singles.tile([128, 2 * T_SLABS], bf)
            ACCD = singles.tile([128, n_accd], f32)
            M2 = m2_pool.tile([2, 512], f32)
            M2S = singles.tile([2, 512], f32)

            # trigger the ACT table load at t~0 via a dependency-free dummy
            DUM = singles.tile([128, 1], f32)
            nc.gpsimd.memset(DUM[:], 0.0)
            nc.scalar.activation(
                DUM[:], DUM[:], mybir.ActivationFunctionType.Exp, bias=0.0, scale=0.0
            )
            # few large DMAs (per-DMA descriptor cost dominates), but split B
            # so the first groups' columns land before the bulk transfer ends
            nc.sync.dma_start(B[:, 0:2048], b_src[:, 0:2048])
            nc.sync.dma_start(A[:], a_src)
            nc.sync.dma_start(LOGS[:], logs_src)
            nc.sync.dma_start(Hh[:], h_src)
            nc.sync.dma_start(B[:, 2048:6144], b_src[:, 2048:6144])
            nc.sync.dma_start(B[:, 6144:N], b_src[:, 6144:N])
            nc.sync.dma_start(R[:, 0:4608], r_src[:, 0:4608].broadcast_to((128, 4608)))
            nc.sync.dma_start(R[:, 4608:N], r_src[:, 4608:N].broadcast_to((128, 4608)))

            mm_i = 0
            for gi, (t, mb0, nb, mlo, mhi, parity) in enumerate(groups):
                lhsT = A[:, t * 128:(t + 1) * 128]
                width = nb * 128
                c0 = mb0 * 128
                pool_g = psA_pool if parity == 0 else psB_pool
                pt = pool_g.tile(
                    [128, GROUP_CAPS[parity] * 128], f32, tag=f"ps{parity}"
                )
                for q0 in range(0, width, 512):
                    qw = min(512, width - q0)
                    nc.tensor.matmul(
                        pt[:, q0:q0 + qw],
                        lhsT=lhsT,
                        rhs=B[:, c0 + q0:c0 + q0 + qw],
                        start=True,
                        stop=True,
                    )
                T = t_pool.tile([128, max(GROUP_CAPS) * 128], bf, tag="T")
                nc.scalar.activation(
                    T[:, 0:width],
                    pt[:, 0:width],
                    mybir.ActivationFunctionType.Exp,
                    bias=LOGS[:, t:t + 1],
                    scale=1.0,
                    accum_out=ACCD[:, gi:gi + 1],
                )
                if mhi > mlo:
                    o0 = mlo * 128
                    w = (mhi - mlo) * 128
                    TR = t_pool.tile([128, max(GROUP_CAPS) * 128], bf, tag="TR")
                    nc.vector.tensor_tensor(
                        TR[:, 0:w],
                        T[:, o0:o0 + w],
                        R[:, c0 + o0:c0 + o0 + w],
                        mybir.AluOpType.mult,
                    )
                    for q in range(0, w, 512):
                        qw = min(512, w - q)
                        nc.tensor.matmul(
                            M2[:, 0:qw],
                            lhsT=Hh[:, 2 * t:2 * t + 2],
                            rhs=TR[:, q:q + qw],
                            start=(mm_i == 0),
                            stop=(mm_i == n_mir - 1),
                            skip_group_check=True,
                        )
                        mm_i += 1

            assert mm_i == n_mir
            nc.vector.tensor_copy(M2S[:], M2[:])
            nc.sync.dma_start(accd_out, ACCD[:])
            nc.sync.dma_start(m2_out, M2S[:])

    nc.compile()
    return nc


def _limbs3(x):
    x = np.asarray(x, np.float64)
    l1 = x.astype(BF16)
    r = x - l1.astype(np.float64)
    l2 = r.astype(BF16)
    r -= l2.astype(np.float64)
    l3 = r.astype(BF16)
    return l1, l2, l3


def _limbs2(x):
    x = np.asarray(x, np.float64)
    l1 = x.astype(BF16)
    l2 = (x - l1.astype(np.float64)).astype(BF16)
    return l1, l2


def _prep_inputs(input, image):
    s = np.asarray(input, np.float32).reshape(N)
    img = np.asarray(image, np.float32).reshape(3, N)
    yy, xx = np.meshgrid(
        np.arange(H, dtype=np.float32), np.arange(W, dtype=np.float32), indexing="ij"
    )
    pos = np.stack([xx, yy], -1).reshape(N, 2) / np.float32(SIGMA_XY)
    feat = np.concatenate([pos, img.T / np.float32(SIGMA_RGB)], 1).astype(np.float32)
    sq = (feat * feat).sum(1, dtype=np.float32).astype(np.float32)

    fA, fB, fC = _limbs3(feat.T)
    sq1, sq2, sq3 = _limbs3(sq)
    lp = np.maximum(np.log1p(-s.astype(np.float64)), -500.0)
    t1, t2, t3 = _limbs3(-0.5 * sq.astype(np.float64) + lp)
    half = np.full(N, -0.5, BF16)
    one = np.ones(N, BF16)
    a = np.concatenate(
        [fA, fA, fB, fA, fC, fB, sq1[None], sq2[None], sq3[None],
         one[None], one[None], one[None]], axis=0).astype(BF16)
    b = np.concatenate(
        [fA, fB, fA, fC, fA, fB, half[None], half[None], half[None],
         t1[None], t2[None], t3[None]], axis=0).astype(BF16)
    s64 = s.astype(np.float64)
    with np.errstate(divide="ignore"):
        logs = np.maximum(np.log(s64), -500.0).astype(np.float32)
    r_full = np.minimum(s64 / np.maximum(1.0 - s64, 1e-300), 1e30).astype(BF16)
    h_full = np.minimum((1.0 - s64) / np.maximum(s64, 1e-300), 1e30)

    in_maps = []
    for k in range(N_CORES):
        own = [(k + 8 * t) % NSLAB for t in range(T_SLABS)]
        rot = [(k + m) % NSLAB for m in range(NSLAB)]
        rows = np.concatenate([np.arange(a0 * 128, (a0 + 1) * 128) for a0 in own])
        cols = np.concatenate([np.arange(m0 * 128, (m0 + 1) * 128) for m0 in rot])
        h1, h2 = _limbs2(h_full[rows])          # [1152] each
        h_packed = np.stack([h1.reshape(T_SLABS, 128), h2.reshape(T_SLABS, 128)], 1)
        # h_src[:, 2t] = limb1 of slab t, h_src[:, 2t+1] = limb2
        h_arr = np.ascontiguousarray(h_packed.reshape(T_SLABS * 2, 128).T.astype(BF16))
        in_maps.append(
            {
                "a_src": np.ascontiguousarray(a[:, rows]),
                "b_src": np.ascontiguousarray(b[:, cols]),
                "logs_src": np.ascontiguousarray(logs[rows].reshape(T_SLABS, 128).T),
                "r_src": np.ascontiguousarray(r_full[cols])[None, :],
                "h_src": h_arr,
            }
        )
    return in_maps


def _run(in_maps, **kwargs):
    if "nc" not in _cached:
        _cached["nc"] = _build_module()
    return bass_utils.run_bass_kernel_spmd(
        _cached["nc"], in_maps, core_ids=list(range(N_CORES)), **kwargs
    )


def kernel(input, image):
    assert input.shape == (1, 1, H, W) and image.shape == (1, 3, H, W)
    in_maps = _prep_inputs(input, image)
    res = _run(in_maps)
    total = 0.0
    for k in range(N_CORES):
        r = res.results[k]
        total += r["accd_out"].sum(dtype=np.float64)
        total += r["m2_out"].sum(dtype=np.float64)
    return np.array(total / N, dtype=np.float32)
